# revision 1
# baseline (speedup 1.0000x reference)
"""Trainium2 Bass kernel for nn_AttachmentPredictor.

Pipeline (per core, data-parallel over batch; 32 batches/core):
  x is pre-transposed on host to feature-major xT [D=1024, rows=32*256].
  stage1: head projection, feature-major psum[jt] += Wh[dk,jt] @ xT[dk, :]
  bias:   per-batch prep/child projections, feature-major [512, 32]
  tanh(Y1 + bias) -> c1, two 256-col segments per tile (per-batch bias)
  stage2/3: hidden layers, feature-major, tanh -> c2, c3
  scorer: [1,512] psum rows of scores via M=1 matmuls
  epilogue: reshape scores to [32, 256], exp(scores + logmask) with
  accumulated row sums, normalize, DMA out [32, 254].

Matmuls run as float32r (TF32-like, full PE rate) or bfloat16 per OPTS.
"""

import ml_dtypes
import numpy as np

import concourse.bass as bass
import concourse.mybir as mybir
import concourse.tile as tile
from concourse import bass_utils
from concourse.bass import ts

F32 = mybir.dt.float32
F32R = mybir.dt.float32r
BF16 = mybir.dt.bfloat16
AF = mybir.ActivationFunctionType

B, S, D, P = 256, 256, 1024, 512
NCORES = 8
BC = B // NCORES            # 32 batches per core
ROWS = BC * S               # 8192 rows per core
NBLK = ROWS // 512          # 16 blocks of 512 rows (2 batches each)
KD = D // 128               # 8 k-tiles over D
KP = P // 128               # 4 k-tiles over P
EPS = 1e-7
NEG = -1e9

OPTS = {
    "s1_dtype": "f32r",   # dtype for stage-1 x and Wh: "f32r" | "bf16"
    "mm_dtype": "f32r",   # dtype for stages 2/3, scorer, bias: "f32r" | "bf16"
    "group": 1,           # blocks processed jointly (lhsT back-to-back reuse)
    "xr_bufs": 4,
    "c_bufs": 10,
    "ps_bufs": 8,
}

_DT = {"f32r": F32R, "bf16": BF16, "f32": F32, "f16": mybir.dt.float16}
_NPDT = {"f32r": np.float32, "bf16": ml_dtypes.bfloat16, "f32": np.float32,
         "f16": np.float16}


# ---------------------------------------------------------------------------
# walrus in this container accepts at most ONE sync wait per instruction;
# split extra waits onto preceding NoOps on the same engine.
def _split_waits(nc, maxw=1):
    ctr = 0
    for f in nc.m.functions:
        for blk in f.blocks:
            insts = blk.instructions
            newlist = []
            changed = False
            for inst in insts:
                si = inst.sync_info
                if si is not None and len(si.on_wait) > maxw:
                    waits = list(si.on_wait)
                    keep = waits[len(waits) - maxw:]
                    extra = waits[: len(waits) - maxw]
                    for j in range(0, len(extra), maxw):
                        ctr += 1
                        newlist.append(
                            mybir.InstNoOp(
                                name=f"waitsplit-{ctr}",
                                engine=inst.engine,
                                ins=[],
                                outs=[],
                                sync_info=mybir.SyncInfo(
                                    on_wait=extra[j: j + maxw], on_update=[]
                                ),
                            )
                        )
                    inst.sync_info = mybir.SyncInfo(
                        on_wait=keep, on_update=list(si.on_update)
                    )
                    changed = True
                newlist.append(inst)
            if changed:
                insts[:] = newlist


# ---------------------------------------------------------------------------
def _build(opts=None, reps=1):
    opts = dict(OPTS, **(opts or {}))
    nc = bass.Bass("TRN2", target_bir_lowering=False, debug=False)

    S1DT = _DT[opts["s1_dtype"]]
    MMDT = _DT[opts["mm_dtype"]]
    G = opts["group"]
    assert NBLK % G == 0

    # All inputs arrive host-cast to the matmul dtypes (f32r shares the
    # f32 byte layout - the PE rounds on read), so plain HWDGE DMAs suffice.
    s1_dma = nc.sync
    mm_dma = nc.sync

    xT_d = nc.dram_tensor("xT", [NBLK, 128, KD * 512], S1DT,
                          kind="ExternalInput").ap()
    xp_d = nc.dram_tensor("xprep", [D, BC], S1DT, kind="ExternalInput").ap()
    xc_d = nc.dram_tensor("xchild", [D, BC], S1DT, kind="ExternalInput").ap()
    wh_d = nc.dram_tensor("wh", [D, P], S1DT, kind="ExternalInput").ap()
    wp_d = nc.dram_tensor("wp", [D, P], S1DT, kind="ExternalInput").ap()
    wc_d = nc.dram_tensor("wc", [D, P], S1DT, kind="ExternalInput").ap()
    w0_d = nc.dram_tensor("w0", [P, P], MMDT, kind="ExternalInput").ap()
    w1_d = nc.dram_tensor("w1", [P, P], MMDT, kind="ExternalInput").ap()
    sc_d = nc.dram_tensor("scT", [128, KP], MMDT, kind="ExternalInput").ap()
    lm_d = nc.dram_tensor("lmask", [BC, S], F32, kind="ExternalInput").ap()
    out_d = nc.dram_tensor("out", [BC, S - 2], F32, kind="ExternalOutput").ap()

    with tile.TileContext(nc) as tc:
        with (
            tc.tile_pool(name="consts", bufs=1) as consts,
            tc.tile_pool(name="ssb", bufs=3) as spool,
            tc.tile_pool(name="xr", bufs=opts["xr_bufs"]) as xpool,
            tc.tile_pool(name="acts", bufs=opts["c_bufs"]) as cpool,
            tc.tile_pool(name="ps", bufs=opts["ps_bufs"], space="PSUM") as pspool,
            tc.tile_pool(name="epi", bufs=1) as epi,
            tc.tile_pool(name="dram", bufs=1, space="DRAM") as dpool,
        ):
            # ---- constants -------------------------------------------------
            def load_packed(dram, k, n, dt, dma, tag):
                t = consts.tile([128, k * n], dt, tag=tag)
                dma.dma_start(
                    t[:].rearrange("p (k n) -> p k n", n=n),
                    dram.rearrange("(k p) n -> p k n", p=128),
                )
                return t

            if opts.get("marker"):
                mk = consts.tile([1, 4], F32, tag="marker")
                nc.gpsimd.memset(mk[:], float(opts["marker"]))
            wh_t = []
            for dk in range(KD):
                wt = consts.tile([128, P], S1DT, tag=f"wh{dk}", name=f"wh_t{dk}")
                s1_dma.dma_start(wt[:], wh_d[dk * 128: (dk + 1) * 128, :])
                wh_t.append(wt)
            xp_r = load_packed(xp_d, KD, BC, S1DT, mm_dma, "xp")
            xc_r = load_packed(xc_d, KD, BC, S1DT, mm_dma, "xc")
            sc_r = consts.tile([128, KP], MMDT, tag="sc")
            mm_dma.dma_start(sc_r[:], sc_d[:])

            # ---- per-batch bias, feature-major [128 j, 32 b] per j-tile ----
            # wp/wc are streamed one [128, 512] d-tile at a time.
            psbs = [pspool.tile([128, BC], F32, tag="ps", name=f"psb_{jt}")
                    for jt in range(KP)]
            for i, (xs, w_d) in enumerate(((xp_r, wp_d), (xc_r, wc_d))):
                for dk in range(KD):
                    wst = spool.tile([128, P], S1DT, tag="wst")
                    mm_dma.dma_start(wst[:], w_d[dk * 128: (dk + 1) * 128, :])
                    for jt in range(KP):
                        nc.tensor.matmul(
                            psbs[jt][:],
                            wst[:, jt * 128: (jt + 1) * 128],
                            xs[:, dk * BC: (dk + 1) * BC],
                            start=(i == 0 and dk == 0),
                            stop=(i == 1 and dk == KD - 1),
                        )
            bias_fm = []
            for jt in range(KP):
                bf = consts.tile([128, BC], F32, tag=f"bias{jt}")
                nc.vector.tensor_copy(bf[:], psbs[jt][:])
                bias_fm.append(bf)

            w0_r = load_packed(w0_d, KP, P, MMDT, mm_dma, "w0")
            w1_r = load_packed(w1_d, KP, P, MMDT, mm_dma, "w1")

            # ---- main loop: groups of G blocks (512 rows each) -------------
            for _rep in range(reps):
                for g in range(NBLK // G):
                    blks = [g * G + i for i in range(G)]
                    xrs = []
                    for blk in blks:
                        xr = xpool.tile([128, KD * 512], S1DT, tag="xr")
                        hw = KD * 512 // 2
                        for h in range(2):
                            s1_dma.dma_start(
                                xr[:, h * hw: (h + 1) * hw],
                                xT_d[blk, :, h * hw: (h + 1) * hw],
                            )
                        xrs.append(xr)

                    # stage 1
                    c1 = [[None] * KP for _ in blks]
                    for jt in range(KP):
                        pss1 = [pspool.tile([128, 512], F32, tag="ps", name=f"ps1_{g}_{jt}_{i}")
                                for i in range(G)]
                        for dk in range(KD):
                            for i in range(G):
                                nc.tensor.matmul(
                                    pss1[i][:],
                                    wh_t[dk][:, jt * 128: (jt + 1) * 128],
                                    xrs[i][:, dk * 512: (dk + 1) * 512],
                                    start=(dk == 0),
                                    stop=(dk == KD - 1),
                                )
                        for i, blk in enumerate(blks):
                            ct = cpool.tile([128, 512], MMDT, tag="c1")
                            for seg in range(2):
                                b = 2 * blk + seg
                                nc.scalar.activation(
                                    ct[:, ts(seg, 256)],
                                    pss1[i][:, ts(seg, 256)],
                                    AF.Tanh,
                                    bias=bias_fm[jt][:, b: b + 1],
                                )
                            c1[i][jt] = ct

                    # stages 2, 3
                    c_in = c1
                    stages23 = () if opts.get("skip_hidden") else ((2, w0_r), (3, w1_r))
                    for stage, w_r in stages23:
                        c_out = [[None] * KP for _ in blks]
                        for qt in range(KP):
                            pss2 = [pspool.tile([128, 512], F32, tag="ps", name=f"ps{stage}_{g}_{qt}_{i}")
                                    for i in range(G)]
                            for jk in range(KP):
                                for i in range(G):
                                    nc.tensor.matmul(
                                        pss2[i][:],
                                        w_r[:, jk * P + qt * 128:
                                            jk * P + (qt + 1) * 128],
                                        c_in[i][jk][:],
                                        start=(jk == 0),
                                        stop=(jk == KP - 1),
                                    )
                            for i in range(G):
                                ct = cpool.tile([128, 512], MMDT,
                                                tag=f"c{stage}")
                                nc.scalar.activation(ct[:], pss2[i][:], AF.Tanh)
                                c_out[i][qt] = ct
                        c_in = c_out

                    # scorer + block-local masked exp-normalization.
                    # The [1, 512] psum holds both batches along the free dim
                    # (cols b*256..), so per-batch [1, 256] slices stay at
                    # partition base 0 (32-alignment rule).
                    for i, blk in enumerate(blks):
                        pss = pspool.tile([1, 512], F32, tag="ps")
                        for qk in range(KP):
                            nc.tensor.matmul(
                                pss[:],
                                sc_r[:, qk: qk + 1],
                                c_in[i][qk][:],
                                start=(qk == 0),
                                stop=(qk == KP - 1),
                            )
                        for bi in range(2):
                            b = 2 * blk + bi
                            lmb = spool.tile([1, S], F32, tag="lmb",
                                             name=f"lmb_{g}_{i}_{bi}")
                            nc.sync.dma_start(lmb[:], lm_d[b: b + 1, :])
                            expin_b = spool.tile([1, S], F32, tag="expin_b",
                                                 name=f"ei_{g}_{i}_{bi}")
                            nc.vector.tensor_add(
                                expin_b[:], pss[0:1, bi * S: (bi + 1) * S],
                                lmb[:],
                            )
                            expm_b = spool.tile([1, S], F32, tag="expm_b",
                                                name=f"em_{g}_{i}_{bi}")
                            sums_b = spool.tile([1, 1], F32, tag="sums_b",
                                                name=f"su_{g}_{i}_{bi}")
                            nc.scalar.activation(expm_b[:], expin_b[:], AF.Exp,
                                                 accum_out=sums_b[:])
                            nc.vector.tensor_scalar_add(
                                sums_b[:], sums_b[:], EPS
                            )
                            recip_b = spool.tile([1, 1], F32, tag="recip_b",
                                                 name=f"re_{g}_{i}_{bi}")
                            nc.vector.reciprocal(recip_b[:], sums_b[:])
                            outv_b = spool.tile([1, S], F32, tag="outv_b",
                                                name=f"ov_{g}_{i}_{bi}")
                            nc.vector.tensor_scalar_mul(
                                outv_b[:], expm_b[:], recip_b[:]
                            )
                            nc.sync.dma_start(
                                out_d[b: b + 1, :], outv_b[:, 0: S - 2]
                            )


    _split_waits(nc)
    return nc


# ---------------------------------------------------------------------------
def _host_prep(x, proj_head, proj_prep, proj_child, hidden_layers, scorer, mask,
               opts=None):
    opts = dict(OPTS, **(opts or {}))
    s1_np = _NPDT[opts["s1_dtype"]]
    mm_np = _NPDT[opts["mm_dtype"]]
    x = np.asarray(x, np.float32)
    mask = np.asarray(mask)
    wh = np.ascontiguousarray(np.asarray(proj_head, s1_np))
    wp = np.ascontiguousarray(np.asarray(proj_prep, s1_np))
    wc = np.ascontiguousarray(np.asarray(proj_child, s1_np))
    hl = np.asarray(hidden_layers, np.float32)
    w0 = np.ascontiguousarray(hl[0].astype(mm_np))
    w1 = np.ascontiguousarray(hl[1].astype(mm_np))
    scT = np.ascontiguousarray(
        np.asarray(scorer, np.float32).reshape(KP, 128).T.astype(mm_np)
    )  # [128, 4]

    in_maps = []
    for c in range(NCORES):
        xb = x[c * BC: (c + 1) * BC]                       # [32, 256, 1024]
        xf = xb.reshape(ROWS, D)                            # [8192, 1024]
        xTc = np.ascontiguousarray(
            xf.reshape(NBLK, 512, KD, 128).transpose(0, 3, 2, 1).astype(s1_np)
        ).reshape(NBLK, 128, KD * 512)
        xpc = np.ascontiguousarray(xb[:, S - 2, :].T.astype(s1_np))  # [1024, 32]
        xcc = np.ascontiguousarray(xb[:, S - 1, :].T.astype(s1_np))  # [1024, 32]
        mb = mask[c * BC: (c + 1) * BC]                    # [32, 256]
        lm = np.full((BC, S), NEG, np.float32)
        lm[:, : S - 2][mb[:, : S - 2]] = 0.0
        in_maps.append(
            {
                "xT": xTc, "xprep": xpc, "xchild": xcc,
                "wh": wh, "wp": wp, "wc": wc, "w0": w0, "w1": w1,
                "scT": scT, "lmask": lm,
            }
        )
    return in_maps


_NC_CACHE = {}


def _get_nc(key="default"):
    if key not in _NC_CACHE:
        _NC_CACHE[key] = _build()
    return _NC_CACHE[key]


def kernel(x, proj_head, proj_prep, proj_child, hidden_layers, scorer, mask):
    in_maps = _host_prep(
        x, proj_head, proj_prep, proj_child, hidden_layers, scorer, mask
    )
    nc = _get_nc()
    res = bass_utils.run_bass_kernel_spmd(
        nc, in_maps, core_ids=list(range(NCORES))
    )
    out = np.concatenate([r["out"] for r in res.results], axis=0)
    return out.astype(np.float32)


if __name__ == "__main__":
    rng = np.random.default_rng(0)
    x = rng.standard_normal((B, S, D)).astype(np.float32)
    u = lambda shp: rng.uniform(-0.05, 0.05, shp).astype(np.float32)
    inputs = dict(
        x=x, proj_head=u((D, P)), proj_prep=u((D, P)), proj_child=u((D, P)),
        hidden_layers=u((2, P, P)), scorer=u((P,)),
        mask=rng.integers(0, 2, (B, S)).astype(bool),
    )
    out = kernel(**inputs)
    print("kernel out", out.shape, out.dtype, out[:2, :4])



# revision 3
# speedup vs baseline: 5.9911x; 5.9911x over previous
"""Trainium2 Bass kernel for nn_AttachmentPredictor (masked-row packed).

Only ~50% of sequence positions survive the mask; the reference zeroes the
rest. Host packs the masked-in rows of each core's batches contiguously
(batches load-balanced across cores), so the device processes ~4096 rows
instead of 8192. Per-row prep/child bias and the per-batch exp-sum
normalization are handled with small one-hot matmuls on the PE:

  stage1:  psum[j, r]  = sum_d wh[d, j] x[d, r]  (+ biasT one-hot matmul)
  c1 = tanh(psum / S);  stages 2, 3 the same with w0, w1
  scorer:  [1, 512] psum per 512-row block; PE-transpose to [128, 4]
  exp -> one-hot segsum matmuls accumulate per-batch sums
  phase 2: recip(sums) broadcast back to rows via one-hot matmul, multiply,
  DMA packed scores out; host scatters into the [B, S-2] zeros.

Each matmul stage runs in bf16 or fp8e4m3+DoubleRow per OPTS.
"""

import ml_dtypes
import numpy as np

import concourse.bass as bass
import concourse.mybir as mybir
import concourse.tile as tile
from concourse import bass_utils
from concourse.bass import ts

F32 = mybir.dt.float32
F32R = mybir.dt.float32r
BF16 = mybir.dt.bfloat16
F8E4 = mybir.dt.float8e4
AF = mybir.ActivationFunctionType
DR = mybir.MatmulPerfMode.DoubleRow

B, S, D, P = 256, 256, 1024, 512
NCORES = 8
BC = B // NCORES            # batches per core
KD = D // 128               # 8 k-tiles over D
KP = P // 128               # 4 k-tiles over P
EPS = 1e-7

OPTS = {
    "s1": "bf16",    # "dr" (fp8e4 DoubleRow) | "bf16" | "f32r"
    "s2": "bf16",
    "s3": "bf16",
    "sc": "bf16",
    "bias": "bf16",  # bias one-hot matmul dtype ("bf16" only for now)
    "xr_bufs": 3,
    "c_bufs": 2,
    "ps_bufs": 5,
    "ps2_bufs": 3,
}

SX = 2.0    # fp8 quant scale for x
SW = 64.0   # fp8 quant scale for weights

_NPDT = {"dr": ml_dtypes.float8_e4m3, "bf16": ml_dtypes.bfloat16,
         "f32r": np.float32}
_BDT = {"dr": F8E4, "bf16": BF16, "f32r": F32R}


def _scales(kind):
    # (x_or_act_scale, w_scale) used when host-quantizing that stage's inputs
    if kind == "dr":
        return SX, SW
    return 1.0, 1.0


# ---------------------------------------------------------------------------
# walrus in this container accepts at most ONE sync wait per instruction;
# split extra waits onto preceding NoOps on the same engine.
def _split_waits(nc, maxw=1):
    ctr = 0
    for f in nc.m.functions:
        for blk in f.blocks:
            insts = blk.instructions
            newlist = []
            changed = False
            for inst in insts:
                si = inst.sync_info
                if si is not None and len(si.on_wait) > maxw:
                    waits = list(si.on_wait)
                    keep = waits[len(waits) - maxw:]
                    extra = waits[: len(waits) - maxw]
                    for j in range(0, len(extra), maxw):
                        ctr += 1
                        newlist.append(
                            mybir.InstNoOp(
                                name=f"waitsplit-{ctr}",
                                engine=inst.engine,
                                ins=[],
                                outs=[],
                                sync_info=mybir.SyncInfo(
                                    on_wait=extra[j: j + maxw], on_update=[]
                                ),
                            )
                        )
                    inst.sync_info = mybir.SyncInfo(
                        on_wait=keep, on_update=list(si.on_update)
                    )
                    changed = True
                newlist.append(inst)
            if changed:
                insts[:] = newlist


# ---------------------------------------------------------------------------
def _build(nrow, opts=None, reps=1):
    opts = dict(OPTS, **(opts or {}))
    nblk = nrow // 512
    d1, d2, d3, dsc = (_BDT[opts[k]] for k in ("s1", "s2", "s3", "sc"))
    dr1, dr2, dr3, drsc = (opts[k] == "dr" for k in ("s1", "s2", "s3", "sc"))
    sx1, sw1 = _scales(opts["s1"])
    _, sw2 = _scales(opts["s2"])
    _, sw3 = _scales(opts["s3"])
    _, swsc = _scales(opts["sc"])

    nc = bass.Bass("TRN2", target_bir_lowering=False, debug=False)

    xT_d = nc.dram_tensor("xT", [nblk, 128, KD * 512], d1,
                          kind="ExternalInput").ap()
    wh_d = nc.dram_tensor("wh", [128, KD * 512], d1, kind="ExternalInput").ap()
    w0_d = nc.dram_tensor("w0", [128, KP * 512], d2, kind="ExternalInput").ap()
    w1_d = nc.dram_tensor("w1", [128, KP * 512], d3, kind="ExternalInput").ap()
    sc_d = nc.dram_tensor("scT", [128, KP], dsc, kind="ExternalInput").ap()
    xp_d = nc.dram_tensor("xprep", [128, KD * BC], BF16,
                          kind="ExternalInput").ap()
    xc_d = nc.dram_tensor("xchild", [128, KD * BC], BF16,
                          kind="ExternalInput").ap()
    wp_d = nc.dram_tensor("wp", [D, P], BF16, kind="ExternalInput").ap()
    wc_d = nc.dram_tensor("wc", [D, P], BF16, kind="ExternalInput").ap()
    ohb_d = nc.dram_tensor("ohb", [BC, nrow], BF16, kind="ExternalInput").ap()
    ohs_d = nc.dram_tensor("ohs", [128, nblk * 4 * BC], BF16,
                           kind="ExternalInput").ap()
    out_d = nc.dram_tensor("out", [nblk, 128, 4], F32,
                           kind="ExternalOutput").ap()

    with tile.TileContext(nc) as tc:
        with (
            tc.tile_pool(name="consts", bufs=1) as consts,
            tc.tile_pool(name="ssb", bufs=3) as spool,
            tc.tile_pool(name="xr", bufs=opts["xr_bufs"]) as xpool,
            tc.tile_pool(name="acts", bufs=opts["c_bufs"]) as cpool,
            tc.tile_pool(name="epi", bufs=2) as epool,
            tc.tile_pool(name="expt", bufs=nblk + 1) as xppool,
            tc.tile_pool(name="ps", bufs=opts["ps_bufs"], space="PSUM") as pspool,
            tc.tile_pool(name="ps2", bufs=opts["ps2_bufs"], space="PSUM") as ps2pool,
        ):
            # ---- constants -------------------------------------------------
            wh_t = consts.tile([128, KD * 512], d1, tag="wh")
            nc.sync.dma_start(wh_t[:], wh_d)
            whv = wh_t[:].rearrange("p (o r) -> p o r", r=512)
            w0_t = consts.tile([128, KP * 512], d2, tag="w0")
            nc.sync.dma_start(w0_t[:], w0_d)
            w0v = w0_t[:].rearrange("p (o r) -> p o r", r=512)
            w1_t = consts.tile([128, KP * 512], d3, tag="w1")
            nc.sync.dma_start(w1_t[:], w1_d)
            w1v = w1_t[:].rearrange("p (o r) -> p o r", r=512)
            sc_t = consts.tile([128, KP], dsc, tag="sc")
            nc.sync.dma_start(sc_t[:], sc_d)
            scv = sc_t[:].rearrange("p (o u) -> p o u", u=1)
            xp_t = consts.tile([128, KD * BC], BF16, tag="xp")
            nc.sync.dma_start(xp_t[:], xp_d)
            xc_t = consts.tile([128, KD * BC], BF16, tag="xc")
            nc.sync.dma_start(xc_t[:], xc_d)
            ohb_t = consts.tile([BC, nrow], BF16, tag="ohb")
            nc.sync.dma_start(ohb_t[:], ohb_d)
            ohs_t = consts.tile([128, nblk * 4 * BC], BF16, tag="ohs")
            nc.sync.dma_start(ohs_t[:], ohs_d)
            ones_t = consts.tile([1, 1], F32, tag="ones")
            nc.vector.memset(ones_t[:], 1.0)

            # ---- per-batch bias biasT[b, j] = (xp^T wp + xc^T wc)[b, j] ----
            psb = ps2pool.tile([BC, 512], F32, tag="ps2", name="psb")
            for i, (xs_t, w_d) in enumerate(((xp_t, wp_d), (xc_t, wc_d))):
                for dk in range(KD):
                    wst = spool.tile([128, P], BF16, tag="wst")
                    nc.sync.dma_start(wst[:], w_d[dk * 128: (dk + 1) * 128, :])
                    nc.tensor.matmul(
                        psb[:],
                        xs_t[:, dk * BC: (dk + 1) * BC],
                        wst[:],
                        start=(i == 0 and dk == 0),
                        stop=(i == 1 and dk == KD - 1),
                    )
            biasT = consts.tile([BC, 512], BF16, tag="biasT")
            nc.scalar.mul(biasT[:], psb[:], sx1 * sw1)

            # ---- main loop -------------------------------------------------
            for _rep in range(reps):
                sums = epool.tile([1, BC], F32, tag="sums", name=f"sums{_rep}")
                nc.vector.memset(sums[:], 0.0)
                expTs = []
                for blk in range(nblk):
                    xr = xpool.tile([128, KD * 512], d1, tag="xr")
                    hw = KD * 512 // 2
                    for h in range(2):
                        nc.sync.dma_start(
                            xr[:, h * hw: (h + 1) * hw],
                            xT_d[blk, :, h * hw: (h + 1) * hw],
                        )
                    xv = xr[:].rearrange("p (o r) -> p o r", r=512)

                    # stage 1 (+ bias) -> c1
                    c1 = cpool.tile([128, KP * 512], d2, tag="c1")
                    c1v = c1[:].rearrange("p (o r) -> p o r", r=512)
                    for jt in range(KP):
                        ps = pspool.tile([128, 512], F32, tag="ps",
                                         name=f"ps1_{_rep}_{blk}_{jt}")
                        if dr1:
                            for o in range(KD // 2):
                                nc.tensor.matmul(
                                    ps[:],
                                    whv[:, 2 * o: 2 * o + 2, ts(jt, 128)],
                                    xv[:, 2 * o: 2 * o + 2, :],
                                    start=(o == 0), stop=False,
                                    perf_mode=DR,
                                )
                        else:
                            for o in range(KD):
                                nc.tensor.matmul(
                                    ps[:],
                                    whv[:, o, ts(jt, 128)],
                                    xv[:, o, :],
                                    start=(o == 0), stop=False,
                                )
                        nc.tensor.matmul(
                            ps[:],
                            biasT[:, ts(jt, 128)],
                            ohb_t[:, blk * 512: (blk + 1) * 512],
                            start=False, stop=True,
                        )
                        nc.scalar.activation(c1v[:, jt, :], ps[:], AF.Tanh,
                                             scale=1.0 / (sx1 * sw1))

                    # stages 2, 3
                    c_in, c_out = c1v, None
                    for stage, wv, drx, dnext, swx in (
                        (2, w0v, dr2, d3, sw2), (3, w1v, dr3, dsc, sw3),
                    ):
                        cn = cpool.tile([128, KP * 512], dnext, tag=f"c{stage}")
                        cnv = cn[:].rearrange("p (o r) -> p o r", r=512)
                        for qt in range(KP):
                            ps = pspool.tile([128, 512], F32, tag="ps",
                                             name=f"ps{stage}_{_rep}_{blk}_{qt}")
                            if drx:
                                for o in range(KP // 2):
                                    nc.tensor.matmul(
                                        ps[:],
                                        wv[:, 2 * o: 2 * o + 2, ts(qt, 128)],
                                        c_in[:, 2 * o: 2 * o + 2, :],
                                        start=(o == 0), stop=(o == KP // 2 - 1),
                                        perf_mode=DR,
                                    )
                            else:
                                for o in range(KP):
                                    nc.tensor.matmul(
                                        ps[:],
                                        wv[:, o, ts(qt, 128)],
                                        c_in[:, o, :],
                                        start=(o == 0), stop=(o == KP - 1),
                                    )
                            nc.scalar.activation(cnv[:, qt, :], ps[:], AF.Tanh,
                                                 scale=1.0 / swx)
                        c_in = cnv

                    # scorer -> [1, 512] psum
                    pss = ps2pool.tile([1, 512], F32, tag="ps2",
                                       name=f"pss_{_rep}_{blk}")
                    if drsc:
                        for o in range(KP // 2):
                            nc.tensor.matmul(
                                pss[:],
                                scv[:, 2 * o: 2 * o + 2, :],
                                c_in[:, 2 * o: 2 * o + 2, :],
                                start=(o == 0), stop=(o == KP // 2 - 1),
                                perf_mode=DR,
                            )
                    else:
                        for o in range(KP):
                            nc.tensor.matmul(
                                pss[:],
                                sc_t[:, o: o + 1],
                                c_in[:, o, :],
                                start=(o == 0), stop=(o == KP - 1),
                            )
                    sv = spool.tile([1, 512], F32, tag="sv",
                                    name=f"sv_{_rep}_{blk}")
                    nc.vector.tensor_copy(sv[:], pss[:])

                    # transpose scores to [128, 4], exp, segmented sums
                    pst = ps2pool.tile([128, 4], F32, tag="ps2",
                                       name=f"pst_{_rep}_{blk}")
                    for t in range(4):
                        nc.tensor.matmul(
                            pst[:, t: t + 1], sv[:, ts(t, 128)], ones_t[:],
                            is_transpose=True,
                        )
                    expT = xppool.tile([128, 4], F32, tag="expT",
                                       name=f"expT_{_rep}_{blk}")
                    nc.scalar.activation(expT[:], pst[:], AF.Exp,
                                         scale=1.0 / swsc)
                    expTb = spool.tile([128, 4], BF16, tag="expTb",
                                       name=f"expTb_{_rep}_{blk}")
                    nc.vector.tensor_copy(expTb[:], expT[:])
                    pseg = ps2pool.tile([1, BC], F32, tag="ps2",
                                        name=f"pseg_{_rep}_{blk}")
                    for t in range(4):
                        nc.tensor.matmul(
                            pseg[:],
                            expTb[:, t: t + 1],
                            ohs_t[:, (blk * 4 + t) * BC: (blk * 4 + t + 1) * BC],
                            start=(t == 0), stop=(t == 3),
                        )
                    nc.vector.tensor_add(sums[:], sums[:], pseg[:])
                    expTs.append(expT)

                # ---- phase 2: normalize + output --------------------------
                nc.vector.tensor_scalar_add(sums[:], sums[:], EPS)
                recip = epool.tile([1, BC], F32, tag="recip",
                                   name=f"recip{_rep}")
                nc.vector.reciprocal(recip[:], sums[:])
                psr = ps2pool.tile([BC, 1], F32, tag="ps2", name=f"psr{_rep}")
                nc.tensor.matmul(psr[:], recip[:], ones_t[:],
                                 is_transpose=True)
                recipT = epool.tile([BC, 1], BF16, tag="recipT",
                                    name=f"recipT{_rep}")
                nc.vector.tensor_copy(recipT[:], psr[:])
                for blk in range(nblk):
                    prr = ps2pool.tile([128, 4], F32, tag="ps2",
                                       name=f"prr_{_rep}_{blk}")
                    for t in range(4):
                        nc.tensor.matmul(
                            prr[:, t: t + 1],
                            ohb_t[:, blk * 512 + t * 128: blk * 512 + (t + 1) * 128],
                            recipT[:],
                            start=True, stop=True,
                        )
                    outv = spool.tile([128, 4], F32, tag="outv",
                                      name=f"outv_{_rep}_{blk}")
                    nc.vector.tensor_mul(outv[:], expTs[blk][:], prr[:])
                    nc.sync.dma_start(out_d[blk], outv[:])

    _split_waits(nc)
    return nc


# ---------------------------------------------------------------------------
def _host_prep(x, proj_head, proj_prep, proj_child, hidden_layers, scorer, mask,
               opts=None):
    opts = dict(OPTS, **(opts or {}))
    x = np.asarray(x, np.float32)
    mask = np.asarray(mask)
    head_mask = mask[:, : S - 2]
    counts = head_mask.sum(axis=1).astype(np.int64)  # [B]

    # balance batches across cores (LPT, capacity BC per core)
    order = np.argsort(-counts, kind="stable")
    core_batches = [[] for _ in range(NCORES)]
    core_rows = np.zeros(NCORES, np.int64)
    for b in order:
        cands = [c for c in range(NCORES) if len(core_batches[c]) < BC]
        c = min(cands, key=lambda c: core_rows[c])
        core_batches[c].append(int(b))
        core_rows[c] += counts[b]
    nrow = int(max(512, ((core_rows.max() + 511) // 512) * 512))
    nblk = nrow // 512

    np1, np2, np3, npsc = (_NPDT[opts[k]] for k in ("s1", "s2", "s3", "sc"))
    sx1, sw1 = _scales(opts["s1"])
    _, sw2 = _scales(opts["s2"])
    _, sw3 = _scales(opts["s3"])
    _, swsc = _scales(opts["sc"])

    wh = np.asarray(proj_head, np.float32)
    hl = np.asarray(hidden_layers, np.float32)
    sc = np.asarray(scorer, np.float32)

    # weight tiles [128, ktiles*512]: element (p, o*512+j) = W[o*128+p, j]*sw
    wh_pk = np.ascontiguousarray(
        (wh * sw1).reshape(KD, 128, P).transpose(1, 0, 2).astype(np1)
    ).reshape(128, KD * P)
    w0_pk = np.ascontiguousarray(
        (hl[0] * sw2).reshape(KP, 128, P).transpose(1, 0, 2).astype(np2)
    ).reshape(128, KP * P)
    w1_pk = np.ascontiguousarray(
        (hl[1] * sw3).reshape(KP, 128, P).transpose(1, 0, 2).astype(np3)
    ).reshape(128, KP * P)
    sc_pk = np.ascontiguousarray(
        (sc * swsc).reshape(KP, 128).T.astype(npsc)
    )  # [128, 4]
    wp_bf = np.asarray(proj_prep, np.float32).astype(ml_dtypes.bfloat16)
    wc_bf = np.asarray(proj_child, np.float32).astype(ml_dtypes.bfloat16)

    in_maps, scatter = [], []
    for c in range(NCORES):
        bs = core_batches[c]
        b_loc, s_idx, g_idx = [], [], []
        for i, gb in enumerate(bs):
            ss = np.nonzero(head_mask[gb])[0]
            b_loc.append(np.full(len(ss), i, np.int64))
            s_idx.append(ss)
            g_idx.append(np.full(len(ss), gb, np.int64))
        b_loc = np.concatenate(b_loc) if b_loc else np.zeros(0, np.int64)
        s_idx = np.concatenate(s_idx) if s_idx else np.zeros(0, np.int64)
        g_idx = np.concatenate(g_idx) if g_idx else np.zeros(0, np.int64)
        T = len(s_idx)

        xg = np.zeros((nrow, D), np.float32)
        xg[:T] = x[g_idx, s_idx]
        xT = np.ascontiguousarray(
            (xg * sx1).reshape(nblk, 512, KD, 128).transpose(0, 3, 2, 1)
            .astype(np1)
        ).reshape(nblk, 128, KD * 512)

        xb = x[np.asarray(bs, np.int64)]                     # [BC, S, D]
        xp_pk = np.ascontiguousarray(
            xb[:, S - 2, :].T.reshape(KD, 128, BC).transpose(1, 0, 2)
            .astype(ml_dtypes.bfloat16)
        ).reshape(128, KD * BC)
        xc_pk = np.ascontiguousarray(
            xb[:, S - 1, :].T.reshape(KD, 128, BC).transpose(1, 0, 2)
            .astype(ml_dtypes.bfloat16)
        ).reshape(128, KD * BC)

        ohb = np.zeros((BC, nrow), np.float32)
        ohb[b_loc, np.arange(T)] = 1.0
        ohs = np.ascontiguousarray(
            ohb.T.reshape(nblk, 4, 128, BC).transpose(2, 0, 1, 3)
        ).reshape(128, nblk * 4 * BC)

        in_maps.append({
            "xT": xT, "wh": wh_pk, "w0": w0_pk, "w1": w1_pk, "scT": sc_pk,
            "xprep": xp_pk, "xchild": xc_pk, "wp": wp_bf, "wc": wc_bf,
            "ohb": ohb.astype(ml_dtypes.bfloat16),
            "ohs": ohs.astype(ml_dtypes.bfloat16),
        })
        scatter.append((g_idx, s_idx))
    return in_maps, scatter, nrow


_NC_CACHE = {}


def _get_nc(nrow, opts=None, reps=1):
    key = (nrow, reps, tuple(sorted((dict(OPTS, **(opts or {}))).items())))
    if key not in _NC_CACHE:
        _NC_CACHE[key] = _build(nrow, opts=opts, reps=reps)
    return _NC_CACHE[key]


def kernel(x, proj_head, proj_prep, proj_child, hidden_layers, scorer, mask,
           opts=None):
    in_maps, scatter, nrow = _host_prep(
        x, proj_head, proj_prep, proj_child, hidden_layers, scorer, mask,
        opts=opts,
    )
    nc = _get_nc(nrow, opts=opts)
    res = bass_utils.run_bass_kernel_spmd(
        nc, in_maps, core_ids=list(range(NCORES))
    )
    out = np.zeros((B, S - 2), np.float32)
    for c in range(NCORES):
        vals = res.results[c]["out"]          # [nblk, 128, 4]
        flat = vals.transpose(0, 2, 1).reshape(-1)  # row-major packed
        g_idx, s_idx = scatter[c]
        out[g_idx, s_idx] = flat[: len(g_idx)]
    return out


if __name__ == "__main__":
    rng = np.random.default_rng(0)
    x = rng.standard_normal((B, S, D)).astype(np.float32)
    u = lambda shp: rng.uniform(-0.05, 0.05, shp).astype(np.float32)
    inputs = dict(
        x=x, proj_head=u((D, P)), proj_prep=u((D, P)), proj_child=u((D, P)),
        hidden_layers=u((2, P, P)), scorer=u((P,)),
        mask=rng.integers(0, 2, (B, S)).astype(bool),
    )
    out = kernel(**inputs)
    print("kernel out", out.shape, out.dtype, out[:2, :4])
